# revision 4
# baseline (speedup 1.0000x reference)
"""Multi-head attention (b=2, n=2048, d=1024, h=16) on 8 TRN2 NeuronCores.

Sharding: data-parallel over batch (2) x tensor-parallel over head-groups (4).
Core c handles batch c//4, heads 4*(c%4)..4*(c%4)+3 (channel rows 256*(c%4)..).
Column-parallel QKV, row-parallel output projection with fine-grained
(128-token) on-device ReduceScatter (bf16) over each 4-core batch group,
writing directly into the output tensor; each core emits its token slices of
the final output which the host reassembles.

Matmul operands are bf16 (PE full rate; fp32 PSUM accumulation); softmax
statistics and normalization run in fp32. Host-side prep is layout-only
(slicing/transpose/dtype): the device receives x^T and weight shards
pre-transposed so every matmul operand is already in its natural
(contraction-on-partition) layout.

Scheduling notes (to keep the in-order PE queue dense and the HAM clock
warm): the out-projection of i-block k-1 and the second QKV pair are
interleaved as small "filler" chunks inside the attention j-loops, the
softmax normalization runs entirely off the PE (DVE reciprocal + GpSimd
partition_broadcast), and the x^T input DMA is chunked per 512-token block
so the first projection matmuls start early.
"""

import sys
from contextlib import ExitStack

_TRN_REPO = "/opt/trn_rl_repo"
if _TRN_REPO not in sys.path:
    sys.path.insert(0, _TRN_REPO)

import ml_dtypes
import numpy as np

import concourse.bass as bass
import concourse.bacc as bacc
import concourse.tile as tile
from concourse import mybir

F32 = mybir.dt.float32
BF16 = mybir.dt.bfloat16

B = 2          # batch
N = 2048       # tokens
D = 1024       # model dim
H = 16         # heads
HD = D // H    # 64 head dim
N_CORES = 8
GROUPS = [[0, 1, 2, 3], [4, 5, 6, 7]]
HPC = 4        # heads per core
CPC = HPC * HD  # 256 channels per core
BW = 512       # attention i-block width (tokens)


def build_program(n=N):
    assert n % BW == 0
    nj = n // 128           # key tiles
    nblk = n // BW          # i blocks
    ntile = n // 128        # 128-token outproj/RS tiles

    nc = bacc.Bacc("TRN2", target_bir_lowering=False, debug=False,
                   num_devices=N_CORES)

    # ---- DRAM I/O (per-core shards, host-prepared, bf16) ----
    xt_d = nc.dram_tensor("xt", [D, n], BF16, kind="ExternalInput").ap()
    wqt_d = nc.dram_tensor("wqt", [D, CPC], BF16, kind="ExternalInput").ap()
    wkt_d = nc.dram_tensor("wkt", [D, CPC], BF16, kind="ExternalInput").ap()
    wvt_d = nc.dram_tensor("wvt", [D, CPC], BF16, kind="ExternalInput").ap()
    wot_d = nc.dram_tensor("wot", [CPC, D], BF16, kind="ExternalInput").ap()
    bo_d = nc.dram_tensor("bob", [128, D], F32, kind="ExternalInput").ap()
    # each RS tile hands this core 32 tokens; 16 tiles -> 512 rows
    out_d = nc.dram_tensor("out", [n // 4, D], BF16, kind="ExternalOutput").ap()

    part_d = [nc.dram_tensor(f"part{t}", [128, D], BF16).ap()
              for t in range(ntile)]
    rs_d = [nc.dram_tensor(f"rsc{t}", [32, D], BF16).ap()
            for t in range(ntile)]

    with tile.TileContext(nc) as tc, ExitStack() as octx:
        wpool = octx.enter_context(tc.tile_pool(name="wpool", bufs=1))
        qk_pool = octx.enter_context(tc.tile_pool(name="qk", bufs=1))
        v_pool = octx.enter_context(tc.tile_pool(name="vaug", bufs=1))
        o_pool = octx.enter_context(tc.tile_pool(name="opair", bufs=1))
        xt_pool = octx.enter_context(tc.tile_pool(name="xt", bufs=1))
        st_pool = octx.enter_context(tc.tile_pool(name="stp", bufs=8))
        nrm_pool = octx.enter_context(tc.tile_pool(name="nrm", bufs=4))
        pp_pool = octx.enter_context(tc.tile_pool(name="pp", bufs=8))
        # PSUM banks: st 2x[128,1024]f32 = 4, ot 2x[65,512] = 2, mm 2x[128,512] = 2
        mm_ps = octx.enter_context(tc.tile_pool(name="mmps", bufs=2, space="PSUM"))
        st_ps_pool = octx.enter_context(
            tc.tile_pool(name="stps", bufs=2, space="PSUM"))
        ot_ps = octx.enter_context(tc.tile_pool(name="otps", bufs=2, space="PSUM"))

        # ---- weights (K/Q first: they gate the first matmuls) ----
        def load_w(name, dram, rows, cols):
            nch = rows // 128
            raw = wpool.tile([128, nch * cols], BF16, tag=name, name=name + "_t")
            nc.sync.dma_start(
                raw[:].rearrange("p (c m) -> p c m", c=nch),
                dram.rearrange("(c p) m -> p c m", p=128))
            return raw[:]

        wkt = load_w("wkt", wkt_d, D, CPC)
        wqt = load_w("wqt", wqt_d, D, CPC)

        # ---- x^T, chunked per 512-token block so QKV ic=0 starts early ----
        xt_sb = [xt_pool.tile([128, n], BF16, tag=f"xtr{ch}", name=f"xtr{ch}")
                 for ch in range(8)]
        for cb in range(n // 512):
            for ch in range(8):
                nc.sync.dma_start(
                    xt_sb[ch][:, 512 * cb: 512 * (cb + 1)],
                    xt_d[128 * ch:128 * (ch + 1), 512 * cb: 512 * (cb + 1)])

        wvt = load_w("wvt", wvt_d, D, CPC)
        wot = load_w("wot", wot_d, CPC, D)

        bias_sb = wpool.tile([128, D], F32, tag="bias")
        nc.sync.dma_start(bias_sb[:], bo_d[:])
        # bias/4 folded into each core's partial before the ReduceScatter sum
        qbias = wpool.tile([128, D], F32, tag="qbias")
        nc.vector.tensor_scalar_mul(qbias[:], bias_sb[:], 0.25)

        ones_f = wpool.tile([128, 64], F32, tag="ones_f")
        nc.gpsimd.memset(ones_f[:], 1.0)

        qtp = [qk_pool.tile([128, n], BF16, tag=f"qtp{p}", name=f"qtp{p}")
               for p in range(2)]
        ktp = [qk_pool.tile([128, n], BF16, tag=f"ktp{p}", name=f"ktp{p}")
               for p in range(2)]
        vaug = [v_pool.tile([128, HPC * 65], BF16, tag=f"vaug{j}", name=f"vaug{j}")
                for j in range(nj)]
        opair = [o_pool.tile([128, n], BF16, tag=f"op{p}", name=f"op{p}")
                 for p in range(2)]

        def qkv_chunk(p, wmat, dst, ic):
            ps = mm_ps.tile([128, 512], F32, tag="mm")
            for ch in range(8):
                nc.tensor.matmul(
                    ps[:],
                    wmat[:, ch * 256 + p * 128: ch * 256 + p * 128 + 128],
                    xt_sb[ch][:, 512 * ic: 512 * (ic + 1)],
                    start=(ch == 0), stop=(ch == 7))
            nc.vector.tensor_copy(dst[p][:, 512 * ic: 512 * (ic + 1)], ps[:])

        def qkv_pair(p):
            for (wmat, dst) in ((wkt, ktp), (wqt, qtp)):
                for ic in range(n // 512):
                    qkv_chunk(p, wmat, dst, ic)

        def v_phase():
            for j in range(nj):
                nc.vector.tensor_copy(
                    vaug[j][:].rearrange("p (h m) -> p h m", h=HPC)[:, :, 64:65],
                    ones_f[:].rearrange("p (h m) -> p h m", m=1)[:, 0:HPC, :])
                for half in range(2):
                    ps = mm_ps.tile([128, 512], F32, tag="mm")
                    for ch in range(8):
                        nc.tensor.matmul(
                            ps[:, 0:128],
                            xt_sb[ch][:, 128 * j: 128 * (j + 1)],
                            wvt[:, ch * 256 + 128 * half:
                                ch * 256 + 128 * half + 128],
                            start=(ch == 0), stop=(ch == 7))
                    dst = vaug[j][:].rearrange(
                        "p (h m) -> p h m", h=HPC)[:, 2 * half: 2 * half + 2, 0:64]
                    src = ps[:, 0:128].rearrange("p (h m) -> p h m", h=2)
                    nc.vector.tensor_copy(dst, src)

        scale = float(HD) ** -0.5

        def attn_block(p, ib, fillers=None):
            """Heads 2p,2p+1 for i-block ib. Scores for both heads land in one
            [128,1024] PSUM tile (head-even cols 0-511, head-odd 512-1023) so a
            single FD=1024 exp serves both. `fillers` is a list of callables
            (out-projection / QKV chunks) drained one per j iteration to keep
            the PE queue dense while exps are in flight."""
            i0 = ib * BW
            fillers = list(fillers) if fillers else []
            fi = 0
            ots = [ot_ps.tile([65, BW], F32, tag="ot", name=f"ot{p}_{ib}_{e}")
                   for e in range(2)]
            def emit_av(j, st_sb):
                for e in range(2):
                    nc.tensor.matmul(
                        ots[e][:],
                        vaug[j][:, 65 * (2 * p + e): 65 * (2 * p + e) + 65],
                        st_sb[:, 512 * e: 512 * e + 512],
                        start=(j == 0), stop=(j == nj - 1))

            # AV emitted 2 iterations behind scores/exp so the in-order PE
            # never head-of-line blocks waiting for the current exp.
            pend = []
            for j in range(nj):
                st_ps = st_ps_pool.tile([128, 1024], F32, tag="st")
                for e in range(2):
                    r0 = 64 * e
                    nc.tensor.matmul(
                        st_ps[:, 512 * e: 512 * e + 512],
                        ktp[p][r0:r0 + 64, 128 * j: 128 * (j + 1)],
                        qtp[p][r0:r0 + 64, i0: i0 + BW],
                        start=True, stop=True)
                st_sb = st_pool.tile([128, 1024], BF16, tag="st")
                nc.scalar.activation(
                    st_sb[:], st_ps[:],
                    mybir.ActivationFunctionType.Exp, scale=scale)
                if fi < len(fillers):
                    fillers[fi]()
                    fi += 1
                pend.append((j, st_sb))
                if len(pend) > 2:
                    emit_av(*pend.pop(0))
            for item in pend:
                emit_av(*item)
            while fi < len(fillers):
                fillers[fi]()
                fi += 1
            # softmax normalization, entirely off the PE: denominator row ->
            # reciprocal (DVE) -> broadcast down 64 partitions (GpSimd) -> mul
            for e in range(2):
                dsb = nrm_pool.tile([1, BW], F32, tag="dsb")
                nc.vector.tensor_copy(dsb[:], ots[e][64:65, :])
                rsb = nrm_pool.tile([1, BW], F32, tag="rsb")
                nc.vector.reciprocal_approx_fast(rsb[:], dsb[:])
                rb = nrm_pool.tile([64, BW], F32, tag="rb")
                nc.gpsimd.partition_broadcast(rb[:], rsb[:], channels=64)
                nc.vector.tensor_mul(
                    opair[p][64 * e: 64 * e + 64, i0: i0 + BW],
                    ots[e][0:64, :], rb[:])

        def outproj_chunks(k):
            """Out-projection + per-tile ReduceScatter for i-block k, split
            into 8 filler chunks (one per (tile, oc-half))."""
            chunks = []
            for it in range(BW // 128):
                itg = k * (BW // 128) + it
                for oc in range(2):
                    def chunk(itg=itg, oc=oc):
                        ps = mm_ps.tile([128, 512], F32, tag="mm")
                        for p in range(2):
                            nc.tensor.matmul(
                                ps[:],
                                opair[p][:, 128 * itg: 128 * (itg + 1)],
                                wot[:, 1024 * p + 512 * oc:
                                    1024 * p + 512 * oc + 512],
                                start=(p == 0), stop=(p == 1))
                        pp_sb = pp_pool.tile([128, 512], BF16, tag="pp")
                        nc.vector.tensor_add(
                            pp_sb[:], ps[:], qbias[:, 512 * oc: 512 * oc + 512])
                        nc.sync.dma_start(
                            part_d[itg][:, 512 * oc: 512 * oc + 512], pp_sb[:])
                        if oc == 1:
                            nc.gpsimd.collective_compute(
                                "ReduceScatter", mybir.AluOpType.add,
                                replica_groups=GROUPS,
                                ins=[part_d[itg][:]],
                                outs=[rs_d[itg][:]])
                            nc.sync.dma_start(
                                out_d[32 * itg: 32 * itg + 32, :], rs_d[itg][:])
                    chunks.append(chunk)
            return chunks

        # ---- schedule: QKV p1 inside block 0, outproj(k-1) inside block k ----
        qkv_pair(0)
        v_phase()
        qkv1 = [lambda p=p, w=w, d=d, ic=ic: qkv_chunk(p, w, d, ic)
                for (w, d) in ((wkt, ktp), (wqt, qtp)) for ic in range(n // 512)
                for p in (1,)]
        attn_block(0, 0, fillers=qkv1)
        attn_block(1, 0)
        for k in range(1, nblk):
            attn_block(0, k, fillers=outproj_chunks(k - 1))
            attn_block(1, k)
        for chunk in outproj_chunks(nblk - 1):
            chunk()

    nc.compile()
    return nc


def make_in_maps(x, wq, wk, wv, wo, bo):
    """Host-side sharding + layout prep (slices/transposes/dtype only)."""
    bf = ml_dtypes.bfloat16
    x = np.asarray(x, dtype=np.float32)
    bo_b = np.ascontiguousarray(
        np.broadcast_to(np.asarray(bo, np.float32)[None, :], (128, D)))
    wq, wk, wv, wo = (np.asarray(w, np.float32) for w in (wq, wk, wv, wo))
    in_maps = []
    for c in range(N_CORES):
        b, g = divmod(c, 4)
        r0 = CPC * g
        in_maps.append({
            "xt": np.ascontiguousarray(x[b].T.astype(bf)),
            "wqt": np.ascontiguousarray(wq[r0:r0 + CPC, :].T.astype(bf)),
            "wkt": np.ascontiguousarray(wk[r0:r0 + CPC, :].T.astype(bf)),
            "wvt": np.ascontiguousarray(wv[r0:r0 + CPC, :].T.astype(bf)),
            "wot": np.ascontiguousarray(wo[:, r0:r0 + CPC].T.astype(bf)),
            "bob": bo_b,
        })
    return in_maps


_PROG_CACHE = {}


def _get_prog(n=N):
    if n not in _PROG_CACHE:
        _PROG_CACHE[n] = build_program(n)
    return _PROG_CACHE[n]


def run(x, wq, wk, wv, wo, bo, trace=False, trace_cores=None):
    """Run on hardware; returns (output [B,N,D], exec_time_ns or None)."""
    from concourse.bass_utils import run_bass_kernel_spmd

    nc = _get_prog()
    in_maps = make_in_maps(x, wq, wk, wv, wo, bo)
    kw = {}
    if trace:
        kw = dict(trace=True, trace_cores=trace_cores or [0])
    res = run_bass_kernel_spmd(nc, in_maps, list(range(N_CORES)), **kw)
    out = np.empty((B, N, D), dtype=np.float32)
    ntile = N // 128
    for c in range(N_CORES):
        b, g = divmod(c, 4)
        o = np.asarray(res.results[c]["out"], dtype=np.float32)
        for t in range(ntile):
            t0 = 128 * t + 32 * g
            out[b, t0:t0 + 32, :] = o[32 * t: 32 * t + 32]
    return out, res.exec_time_ns


def kernel(x, wq, wk, wv, wo, bo):
    out, _ = run(x, wq, wk, wv, wo, bo)
    return out


# revision 10
# speedup vs baseline: 1.0946x; 1.0946x over previous
"""Multi-head attention (b=2, n=2048, d=1024, h=16) on 8 TRN2 NeuronCores.

Sharding: data-parallel over batch (2) x tensor-parallel over head-groups (4).
Core c handles batch c//4, heads 4*(c%4)..4*(c%4)+3 (channel rows 256*(c%4)..).
Column-parallel QKV, row-parallel output projection with fine-grained
(128-token) on-device ReduceScatter (bf16) over each 4-core batch group,
writing directly into the output tensor; each core emits its token slices of
the final output which the host reassembles.

Matmul operands are bf16 (PE full rate; fp32 PSUM accumulation); softmax
statistics and normalization run in fp32. Host-side prep is layout-only
(slicing/transpose/dtype): the device receives x^T and weight shards
pre-transposed so every matmul operand is already in its natural
(contraction-on-partition) layout.

Scheduling notes (to keep the in-order PE queue dense and the HAM clock
warm): the out-projection of i-block k-1 and the second QKV pair are
interleaved as small "filler" chunks inside the attention j-loops, the
softmax normalization runs entirely off the PE (DVE reciprocal + GpSimd
partition_broadcast), and the x^T input DMA is chunked per 512-token block
so the first projection matmuls start early.
"""

import sys
from contextlib import ExitStack

_TRN_REPO = "/opt/trn_rl_repo"
if _TRN_REPO not in sys.path:
    sys.path.insert(0, _TRN_REPO)

import ml_dtypes
import numpy as np

import concourse.bass as bass
import concourse.bacc as bacc
import concourse.tile as tile
from concourse import mybir

F32 = mybir.dt.float32
BF16 = mybir.dt.bfloat16

B = 2          # batch
N = 2048       # tokens
D = 1024       # model dim
H = 16         # heads
HD = D // H    # 64 head dim
N_CORES = 8
GROUPS = [[0, 1, 2, 3], [4, 5, 6, 7]]
HPC = 4        # heads per core
CPC = HPC * HD  # 256 channels per core
BW = 512       # attention i-block width (tokens)


def build_program(n=N):
    assert n % BW == 0
    nj = n // 128           # key tiles
    nblk = n // BW          # i blocks
    ntile = n // 128        # 128-token outproj/RS tiles

    nc = bacc.Bacc("TRN2", target_bir_lowering=False, debug=False,
                   num_devices=N_CORES)

    # ---- DRAM I/O (per-core shards, host-prepared, bf16) ----
    xt_d = nc.dram_tensor("xt", [D, n], BF16, kind="ExternalInput").ap()
    wqt_d = nc.dram_tensor("wqt", [D, CPC], BF16, kind="ExternalInput").ap()
    wkt_d = nc.dram_tensor("wkt", [D, CPC], BF16, kind="ExternalInput").ap()
    wvt_d = nc.dram_tensor("wvt", [D, CPC], BF16, kind="ExternalInput").ap()
    wot_d = nc.dram_tensor("wot", [CPC, D], BF16, kind="ExternalInput").ap()
    bo_d = nc.dram_tensor("bob", [128, D], F32, kind="ExternalInput").ap()
    # each RS tile hands this core 32 tokens; 16 tiles -> 512 rows
    out_d = nc.dram_tensor("out", [n // 4, D], BF16, kind="ExternalOutput").ap()

    # RS chunks: one per 512-token i-block, except the last block is split in
    # two 256-token halves so the tail barrier is small. Each collective is a
    # 4-core barrier with multi-us overhead, so keep the count low.
    part_d = [nc.dram_tensor(f"part{k}", [BW, D], BF16).ap()
              for k in range(nblk)]
    rs_d = [nc.dram_tensor(f"rsc{k}", [128, D], BF16).ap()
            for k in range(nblk - 1)]
    rs_d += [nc.dram_tensor(f"rsc3{h}", [64, D], BF16).ap() for h in range(2)]

    with tile.TileContext(nc) as tc, ExitStack() as octx:
        wpool = octx.enter_context(tc.tile_pool(name="wpool", bufs=1))
        qk_pool = octx.enter_context(tc.tile_pool(name="qk", bufs=1))
        v_pool = octx.enter_context(tc.tile_pool(name="vaug", bufs=1))
        o_pool = octx.enter_context(tc.tile_pool(name="opair", bufs=1))
        xt_pool = octx.enter_context(tc.tile_pool(name="xt", bufs=1))
        st_pool = octx.enter_context(tc.tile_pool(name="stp", bufs=8))
        nrm_pool = octx.enter_context(tc.tile_pool(name="nrm", bufs=4))
        pp_pool = octx.enter_context(tc.tile_pool(name="pp", bufs=8))
        # PSUM banks: st 2x[128,1024]f32 = 4, ot 2x[65,512] = 2, mm 2x[128,512] = 2
        mm_ps = octx.enter_context(tc.tile_pool(name="mmps", bufs=2, space="PSUM"))
        st_ps_pool = octx.enter_context(
            tc.tile_pool(name="stps", bufs=2, space="PSUM"))
        ot_ps = octx.enter_context(tc.tile_pool(name="otps", bufs=2, space="PSUM"))

        # ---- weights (K/Q first: they gate the first matmuls) ----
        def load_w(name, dram, rows, cols):
            nch = rows // 128
            raw = wpool.tile([128, nch * cols], BF16, tag=name, name=name + "_t")
            nc.sync.dma_start(
                raw[:].rearrange("p (c m) -> p c m", c=nch),
                dram.rearrange("(c p) m -> p c m", p=128))
            return raw[:]

        wkt = load_w("wkt", wkt_d, D, CPC)
        wqt = load_w("wqt", wqt_d, D, CPC)

        # ---- x^T, chunked per 512-token block so QKV ic=0 starts early ----
        xt_sb = [xt_pool.tile([128, n], BF16, tag=f"xtr{ch}", name=f"xtr{ch}")
                 for ch in range(8)]
        for cb in range(n // 512):
            for ch in range(8):
                nc.sync.dma_start(
                    xt_sb[ch][:, 512 * cb: 512 * (cb + 1)],
                    xt_d[128 * ch:128 * (ch + 1), 512 * cb: 512 * (cb + 1)])

        wvt = load_w("wvt", wvt_d, D, CPC)
        wot = load_w("wot", wot_d, CPC, D)

        bias_sb = wpool.tile([128, D], F32, tag="bias")
        nc.sync.dma_start(bias_sb[:], bo_d[:])
        # bias/4 folded into each core's partial before the ReduceScatter sum
        qbias = wpool.tile([128, D], F32, tag="qbias")
        nc.vector.tensor_scalar_mul(qbias[:], bias_sb[:], 0.25)

        ones_f = wpool.tile([128, 64], F32, tag="ones_f")
        nc.gpsimd.memset(ones_f[:], 1.0)
        ones1 = wpool.tile([1, 64], BF16, tag="ones1")
        nc.vector.tensor_copy(ones1[:], ones_f[0:1, :])

        qtp = [qk_pool.tile([128, n], BF16, tag=f"qtp{p}", name=f"qtp{p}")
               for p in range(2)]
        ktp = [qk_pool.tile([128, n], BF16, tag=f"ktp{p}", name=f"ktp{p}")
               for p in range(2)]
        vaug = [v_pool.tile([128, HPC * 65], BF16, tag=f"vaug{j}", name=f"vaug{j}")
                for j in range(nj)]
        opair = [o_pool.tile([128, n], BF16, tag=f"op{p}", name=f"op{p}")
                 for p in range(2)]

        def qkv_chunk(p, wmat, dst, ic):
            ps = mm_ps.tile([128, 512], F32, tag="mm")
            for ch in range(8):
                nc.tensor.matmul(
                    ps[:],
                    wmat[:, ch * 256 + p * 128: ch * 256 + p * 128 + 128],
                    xt_sb[ch][:, 512 * ic: 512 * (ic + 1)],
                    start=(ch == 0), stop=(ch == 7))
            nc.vector.tensor_copy(dst[p][:, 512 * ic: 512 * (ic + 1)], ps[:])

        def qkv_pair(p):
            for (wmat, dst) in ((wkt, ktp), (wqt, qtp)):
                for ic in range(n // 512):
                    qkv_chunk(p, wmat, dst, ic)

        def v_phase():
            for j in range(nj):
                nc.vector.tensor_copy(
                    vaug[j][:].rearrange("p (h m) -> p h m", h=HPC)[:, :, 64:65],
                    ones_f[:].rearrange("p (h m) -> p h m", m=1)[:, 0:HPC, :])
                for half in range(2):
                    ps = mm_ps.tile([128, 512], F32, tag="mm")
                    for ch in range(8):
                        nc.tensor.matmul(
                            ps[:, 0:128],
                            xt_sb[ch][:, 128 * j: 128 * (j + 1)],
                            wvt[:, ch * 256 + 128 * half:
                                ch * 256 + 128 * half + 128],
                            start=(ch == 0), stop=(ch == 7))
                    dst = vaug[j][:].rearrange(
                        "p (h m) -> p h m", h=HPC)[:, 2 * half: 2 * half + 2, 0:64]
                    src = ps[:, 0:128].rearrange("p (h m) -> p h m", h=2)
                    nc.vector.tensor_copy(dst, src)

        scale = float(HD) ** -0.5

        def attn_block(p, ib, fillers=None):
            """Heads 2p,2p+1 for i-block ib. Scores for both heads land in one
            [128,1024] PSUM tile (head-even cols 0-511, head-odd 512-1023) so a
            single FD=1024 exp serves both. `fillers` is a list of callables
            (out-projection / QKV chunks) drained one per j iteration to keep
            the PE queue dense while exps are in flight."""
            i0 = ib * BW
            fillers = list(fillers) if fillers else []
            fi = 0
            ots = [ot_ps.tile([65, BW], F32, tag="ot", name=f"ot{p}_{ib}_{e}")
                   for e in range(2)]
            def emit_av(j, st_sb):
                for e in range(2):
                    nc.tensor.matmul(
                        ots[e][:],
                        vaug[j][:, 65 * (2 * p + e): 65 * (2 * p + e) + 65],
                        st_sb[:, 512 * e: 512 * e + 512],
                        start=(j == 0), stop=(j == nj - 1))

            # AV emitted 2 iterations behind scores/exp so the in-order PE
            # never head-of-line blocks waiting for the current exp.
            pend = []
            for j in range(nj):
                st_ps = st_ps_pool.tile([128, 1024], F32, tag="st")
                for e in range(2):
                    r0 = 64 * e
                    nc.tensor.matmul(
                        st_ps[:, 512 * e: 512 * e + 512],
                        ktp[p][r0:r0 + 64, 128 * j: 128 * (j + 1)],
                        qtp[p][r0:r0 + 64, i0: i0 + BW],
                        start=True, stop=True)
                st_sb = st_pool.tile([128, 1024], BF16, tag="st")
                nc.scalar.activation(
                    st_sb[:], st_ps[:],
                    mybir.ActivationFunctionType.Exp, scale=scale)
                if fi < len(fillers):
                    fillers[fi]()
                    fi += 1
                pend.append((j, st_sb))
                if len(pend) > 2:
                    emit_av(*pend.pop(0))
            for item in pend:
                emit_av(*item)
            while fi < len(fillers):
                fillers[fi]()
                fi += 1
            # softmax normalization: reciprocal of the denominator row (read
            # straight from PSUM), broadcast down 64 partitions via a tiny
            # ones-stationary matmul, then scale the AV block.
            for e in range(2):
                dsb = nrm_pool.tile([1, BW], F32, tag="dsb")
                nc.vector.tensor_copy(dsb[:], ots[e][64:65, :])
                rsb = nrm_pool.tile([1, BW], F32, tag="rsb")
                nc.vector.reciprocal_approx_fast(rsb[:], dsb[:])
                rsr = nrm_pool.tile([1, BW], BF16, tag="rsr")
                nc.vector.tensor_copy(rsr[:], rsb[:])
                bps = mm_ps.tile([128, 512], F32, tag="mm")
                nc.tensor.matmul(bps[0:64, :], ones1[:], rsr[:],
                                 start=True, stop=True)
                bsb = nrm_pool.tile([64, BW], F32, tag="bsb")
                nc.vector.tensor_copy(bsb[:], bps[0:64, :])
                nc.vector.tensor_mul(
                    opair[p][64 * e: 64 * e + 64, i0: i0 + BW],
                    ots[e][0:64, :], bsb[:])

        def post_rs(k, half=None):
            """ReduceScatter a chunk of block k's partials, then copy the
            reduced slice to the output (collectives can't write IO tensors)."""
            if half is None:
                src, dst, rows, o0 = part_d[k][:], rs_d[k], 128, 128 * k
            else:
                src = part_d[3][256 * half: 256 * (half + 1), :]
                dst, rows, o0 = rs_d[3 + half], 64, 384 + 64 * half
            nc.gpsimd.collective_compute(
                "ReduceScatter", mybir.AluOpType.add, replica_groups=GROUPS,
                ins=[src], outs=[dst[:]])
            nc.sync.dma_start(out_d[o0: o0 + rows, :], dst[:])

        def outproj_chunks(k):
            """Out-projection for i-block k in 8 filler chunks (one per
            (tile, oc-half)); ReduceScatter posts ride the chunk that
            completes each RS input."""
            chunks = []
            for it in range(BW // 128):
                itg = k * (BW // 128) + it
                for oc in range(2):
                    def chunk(k=k, it=it, itg=itg, oc=oc):
                        ps = mm_ps.tile([128, 512], F32, tag="mm")
                        for p in range(2):
                            nc.tensor.matmul(
                                ps[:],
                                opair[p][:, 128 * itg: 128 * (itg + 1)],
                                wot[:, 1024 * p + 512 * oc:
                                    1024 * p + 512 * oc + 512],
                                start=(p == 0), stop=(p == 1))
                        pp_sb = pp_pool.tile([128, 512], BF16, tag="pp")
                        nc.vector.tensor_add(
                            pp_sb[:], ps[:], qbias[:, 512 * oc: 512 * oc + 512])
                        nc.sync.dma_start(
                            part_d[k][128 * it: 128 * (it + 1),
                                      512 * oc: 512 * oc + 512], pp_sb[:])
                        if oc == 1:
                            if k < 3 and it == 3:
                                post_rs(k)
                            elif k == 3 and it in (1, 3):
                                post_rs(k, half=it // 2)
                    chunks.append(chunk)
            return chunks

        # ---- schedule: QKV p1 inside block 0, outproj(k-1) inside block k ----
        qkv_pair(0)
        v_phase()
        qkv1 = [lambda p=p, w=w, d=d, ic=ic: qkv_chunk(p, w, d, ic)
                for (w, d) in ((wkt, ktp), (wqt, qtp)) for ic in range(n // 512)
                for p in (1,)]
        attn_block(0, 0, fillers=qkv1)
        attn_block(1, 0)
        for k in range(1, nblk):
            attn_block(0, k, fillers=outproj_chunks(k - 1))
            attn_block(1, k)
        for chunk in outproj_chunks(nblk - 1):
            chunk()

    nc.compile()
    return nc


def make_in_maps(x, wq, wk, wv, wo, bo):
    """Host-side sharding + layout prep (slices/transposes/dtype only)."""
    bf = ml_dtypes.bfloat16
    x = np.asarray(x, dtype=np.float32)
    bo_b = np.ascontiguousarray(
        np.broadcast_to(np.asarray(bo, np.float32)[None, :], (128, D)))
    wq, wk, wv, wo = (np.asarray(w, np.float32) for w in (wq, wk, wv, wo))
    in_maps = []
    for c in range(N_CORES):
        b, g = divmod(c, 4)
        r0 = CPC * g
        in_maps.append({
            "xt": np.ascontiguousarray(x[b].T.astype(bf)),
            "wqt": np.ascontiguousarray(wq[r0:r0 + CPC, :].T.astype(bf)),
            "wkt": np.ascontiguousarray(wk[r0:r0 + CPC, :].T.astype(bf)),
            "wvt": np.ascontiguousarray(wv[r0:r0 + CPC, :].T.astype(bf)),
            "wot": np.ascontiguousarray(wo[:, r0:r0 + CPC].T.astype(bf)),
            "bob": bo_b,
        })
    return in_maps


_PROG_CACHE = {}


def _get_prog(n=N):
    if n not in _PROG_CACHE:
        _PROG_CACHE[n] = build_program(n)
    return _PROG_CACHE[n]


def run(x, wq, wk, wv, wo, bo, trace=False, trace_cores=None):
    """Run on hardware; returns (output [B,N,D], exec_time_ns or None)."""
    from concourse.bass_utils import run_bass_kernel_spmd

    nc = _get_prog()
    in_maps = make_in_maps(x, wq, wk, wv, wo, bo)
    kw = {}
    if trace:
        kw = dict(trace=True, trace_cores=trace_cores or [0])
    res = run_bass_kernel_spmd(nc, in_maps, list(range(N_CORES)), **kw)
    out = np.empty((B, N, D), dtype=np.float32)
    for c in range(N_CORES):
        b, g = divmod(c, 4)
        o = np.asarray(res.results[c]["out"], dtype=np.float32)
        # RS chunks: blocks 0-2 full (this core gets 128 tokens each), block 3
        # in two 256-token halves (64 tokens each).
        for k in range(3):
            t0 = BW * k + 128 * g
            out[b, t0:t0 + 128, :] = o[128 * k: 128 * (k + 1)]
        for h in range(2):
            t0 = 3 * BW + 256 * h + 64 * g
            out[b, t0:t0 + 64, :] = o[384 + 64 * h: 384 + 64 * (h + 1)]
    return out, res.exec_time_ns


def kernel(x, wq, wk, wv, wo, bo):
    out, _ = run(x, wq, wk, wv, wo, bo)
    return out
